# revision 1
# baseline (speedup 1.0000x reference)
"""Trainium2 Bass kernel for a 4-layer dense MLP (H=8192), batch=1.

Tensor-parallel over 8 NeuronCores, structured to hide collective latency:

  - Layer 1 (10x8192) is replicated on every core, computed in 8 passes of
    1024 columns; each pass bounces through DRAM into the [128, 64] activation
    layout piecewise, so layer 2 can start contracting on early pieces while
    later pieces are still in flight (layer 2's contraction chunks are ordered
    piece-major via a host-side weight-row permutation).

  - Hidden layers 2-4 are column-sharded (core c owns 1024 columns) and each
    is computed as two 512-column halves A/B. Half A's AllGather fires at
    mid-layer and overlaps half B's compute; the NEXT layer's contraction
    chunks are ordered so chunks 0-31 touch only gathered-A data and 32-63
    only B (again via host-side row permutations), so the next layer starts
    as soon as AG-A lands while AG-B is still in flight. Exposed collective
    latency is nearly zero.

  - The output layer (8192x8) is row-sharded: no collective after layer 4;
    each core emits a partial [8] which the host sums.

  - A dummy AllGather fires at kernel start so the one-time ncfw rendezvous
    barrier overlaps layer-1 compute and weight prefetch.

Compute dtype is fp16 (PSUM accumulation is fp32); measured end-to-end error
vs the f32 reference is ~4e-4 max-rel. Weights stream as contiguous 1 MiB
DMAs into [128, 4096] SBUF tiles (8 contraction chunks x 512 columns each).
"""

import numpy as np

H = 8192
D = 10  # input layer size (4 + 6)
OUT = 8
NCORES = 8
SH = H // NCORES  # 1024 columns per core
HF = 512  # half-width
KC = 64  # contraction chunks of 128 rows
GC = 8  # chunks per DMA group (1 MiB per DMA at 512 cols)
G = KC // GC  # 8 groups per half
WBUFS = 16  # in-flight weight DMA buffers (16 MiB SBUF)

LAST_RESULTS = None
_CACHE = {}


def _perm_piece():
    """Layer-2 input layout: a_sb[p, k] = a1[(k//8)*1024 + p*8 + (k%8)].
    Returns rows[k, p] = global row index feeding chunk k, partition p."""
    k = np.arange(KC)[:, None]
    p = np.arange(128)[None, :]
    return (k // 8) * 1024 + p * 8 + (k % 8)


def _perm_ab():
    """Layer-3/4 input layout: chunks 0-31 hold the gathered A-halves
    (columns [0,512) of every rank), chunks 32-63 the B-halves.
    a_sb[p, k] = half_flat[p*32 + k%32] with half = k//32, and
    half_flat[i] = a_full[(i//512)*1024 + 512*half + i%512]."""
    k = np.arange(KC)[:, None]
    p = np.arange(128)[None, :]
    half = k // 32
    i = p * 32 + (k % 32)
    return (i // 512) * 1024 + 512 * half + (i % 512)


def _build_nc():
    import concourse.bacc as bacc
    import concourse.mybir as mybir
    import concourse.tile as tile

    f16 = mybir.dt.float16
    f32 = mybir.dt.float32
    SIG = mybir.ActivationFunctionType.Sigmoid
    RG = [list(range(NCORES))]

    nc = bacc.Bacc(
        "TRN2", target_bir_lowering=False, debug=False, num_devices=NCORES
    )

    x_d = nc.dram_tensor("x_cat", [D, 1], f16, kind="ExternalInput")
    win_d = nc.dram_tensor("w_in", [D, H], f16, kind="ExternalInput")
    whh_d = nc.dram_tensor("w_hh", [3, 2, G, 128, GC * HF], f16, kind="ExternalInput")
    wout_d = nc.dram_tensor("w_out", [128, 8 * OUT], f16, kind="ExternalInput")
    bias0_d = nc.dram_tensor("bias0", [1, H], f16, kind="ExternalInput")
    bias_d = nc.dram_tensor("bias", [1, 3 * SH], f16, kind="ExternalInput")
    out_d = nc.dram_tensor("out_partial", [1, OUT], f32, kind="ExternalOutput")

    with tile.TileContext(nc) as tc:
        with (
            tc.tile_pool(name="const", bufs=1) as cp,
            tc.tile_pool(name="wpool", bufs=WBUFS) as wp,
            tc.tile_pool(name="apool", bufs=2) as ap,
            tc.tile_pool(name="pspool", bufs=2, space="PSUM") as pp,
            tc.tile_pool(name="dpool", bufs=2, space="DRAM") as dp,
        ):
            one_sb = cp.tile([1, 1], f16)
            nc.gpsimd.memset(one_sb[:], 1.0)

            # Dummy collective: absorbs the one-time ncfw rendezvous barrier
            # concurrently with layer-1 compute + weight prefetch.
            warm_sb = cp.tile([1, 16], f16)
            nc.gpsimd.memset(warm_sb[:], 0.0)
            warm_in = dp.tile([1, 16], f16, tag="warmin")
            warm_out = dp.tile([8, 16], f16, tag="warmout")
            nc.gpsimd.dma_start(warm_in[:], warm_sb[:])
            nc.gpsimd.collective_compute(
                "AllGather",
                mybir.AluOpType.bypass,
                replica_groups=RG,
                ins=[warm_in.opt()],
                outs=[warm_out.opt()],
            )

            x_sb = cp.tile([D, 1], f16)
            nc.scalar.dma_start(x_sb[:], x_d[:])
            win_sb = cp.tile([D, H], f16)
            nc.scalar.dma_start(win_sb[:], win_d[:])
            bias0_sb = cp.tile([1, H], f16)
            nc.scalar.dma_start(bias0_sb[:], bias0_d[:])
            bias_sb = cp.tile([1, 3 * SH], f16)
            nc.scalar.dma_start(bias_sb[:], bias_d[:])
            wout_sb = cp.tile([128, 8 * OUT], f16)
            nc.scalar.dma_start(wout_sb[:], wout_d[:])

            # ---- Layer 1, replicated: 16 passes of 512 cols, piecewise
            # bounce into the [128, 64] piece-major layout for layer 2 ----
            act1_sb = cp.tile([1, H], f16)
            a1_d = dp.tile([16, HF], f16, tag="a1")
            a_sb = ap.tile([128, KC], f16, tag="a")
            for q in range(16):
                h, odd = q // 2, q % 2
                lo = q * HF
                ps1 = pp.tile([1, HF], f32, tag=f"ps{odd}")
                nc.tensor.matmul(
                    ps1[:],
                    x_sb[:],
                    win_sb[:, lo : lo + HF],
                    start=True,
                    stop=False,
                )
                nc.tensor.matmul(
                    ps1[:],
                    one_sb[:],
                    bias0_sb[:, lo : lo + HF],
                    start=False,
                    stop=True,
                )
                nc.scalar.activation(act1_sb[:, lo : lo + HF], ps1[:], SIG)
                nc.scalar.dma_start(
                    a1_d[q : q + 1, :], act1_sb[:, lo : lo + HF]
                )
                nc.scalar.dma_start(
                    a_sb[64 * odd : 64 * odd + 64, 8 * h : 8 * h + 8],
                    a1_d[q].rearrange("(p k) -> p k", p=64),
                )

            # ---- Hidden layers 2-4: two 512-col halves, AG-A at mid-layer ----
            act_half = [None, None]
            for li in range(3):
                a_next = (
                    ap.tile([128, KC], f16, tag="a", name="a_next")
                    if li < 2
                    else None
                )
                for hf in range(2):
                    ps = pp.tile([1, HF], f32, tag=f"ps{hf}")
                    for g in range(G):
                        wt = wp.tile([128, GC * HF], f16, tag="w")
                        nc.sync.dma_start(wt[:], whh_d[li, hf, g])
                        for c in range(GC):
                            k = g * GC + c
                            nc.tensor.matmul(
                                ps[:],
                                a_sb[:, k : k + 1],
                                wt[:, c * HF : (c + 1) * HF],
                                start=(k == 0),
                                stop=False,
                            )
                    nc.tensor.matmul(
                        ps[:],
                        one_sb[:],
                        bias_sb[:, li * SH + hf * HF : li * SH + hf * HF + HF],
                        start=False,
                        stop=True,
                    )
                    act_h = ap.tile([1, HF], f16, tag=f"act{hf}")
                    nc.scalar.activation(act_h[:], ps[:], SIG)
                    act_half[hf] = act_h
                    if li < 2:
                        cc_in = dp.tile([1, HF], f16, tag=f"ccin{hf}")
                        cc_out = dp.tile([128, 32], f16, tag=f"ccout{hf}")
                        nc.gpsimd.dma_start(cc_in[:], act_h[:])
                        nc.gpsimd.collective_compute(
                            "AllGather",
                            mybir.AluOpType.bypass,
                            replica_groups=RG,
                            ins=[cc_in.opt()],
                            outs=[cc_out.opt()],
                        )
                        nc.scalar.dma_start(
                            a_next[:, 32 * hf : 32 * hf + 32], cc_out[:]
                        )
                if li < 2:
                    a_sb = a_next

            # ---- Output layer: row-sharded, partial [8] per core ----
            sc = dp.tile([1, SH], f16, tag="sc")
            nc.scalar.dma_start(sc[:, 0:HF], act_half[0][:])
            nc.scalar.dma_start(sc[:, HF:SH], act_half[1][:])
            a2_sb = ap.tile([128, 8], f16, tag="a2")
            nc.scalar.dma_start(
                a2_sb[:], sc.rearrange("one (p k) -> (one p) k", p=128)
            )
            pso = pp.tile([1, OUT], f32, tag="psO", bufs=1)
            for k in range(8):
                nc.tensor.matmul(
                    pso[:],
                    a2_sb[:, k : k + 1],
                    wout_sb[:, k * OUT : (k + 1) * OUT],
                    start=(k == 0),
                    stop=(k == 7),
                )
            res_sb = ap.tile([1, OUT], f32, tag="res")
            nc.vector.tensor_copy(res_sb[:], pso[:])
            nc.scalar.dma_start(out_d[:], res_sb[:])

    nc.compile()
    return nc


def _prep_inputs(x, s, W_in, W_hh, W_out, b):
    """Shard + fp16-quantize + lay out the inputs for each of the 8 cores."""
    f16 = np.float16
    x_cat = np.concatenate([np.asarray(x), np.asarray(s)]).astype(f16)
    x_cat = np.ascontiguousarray(x_cat.reshape(D, 1))
    Whh16 = np.asarray(W_hh).astype(f16)  # [3, 8192, 8192]
    Win16 = np.ascontiguousarray(np.asarray(W_in).astype(f16))  # [10, 8192]
    Wout16 = np.asarray(W_out).astype(f16)  # [8192, 8]
    b16 = np.asarray(b).astype(f16)  # [5, 8192] (b[4] unused)
    bias0 = np.ascontiguousarray(b16[0].reshape(1, H))

    perms = [_perm_piece(), _perm_ab(), _perm_ab()]  # input layout per layer

    in_maps = []
    for c in range(NCORES):
        cs, ce = c * SH, (c + 1) * SH
        whh_c = np.empty((3, 2, G, 128, GC * HF), f16)
        for li in range(3):
            wcol = Whh16[li][:, cs:ce]  # [8192, 1024]
            wperm = wcol[perms[li]]  # [64, 128, 1024]
            for hf in range(2):
                arr = wperm[:, :, hf * HF : (hf + 1) * HF]  # [64, 128, 512]
                grp = arr.reshape(G, GC, 128, HF).transpose(0, 2, 1, 3)
                whh_c[li, hf] = grp.reshape(G, 128, GC * HF)
        wout_c = np.ascontiguousarray(Wout16[cs:ce, :].reshape(128, 8 * OUT))
        in_maps.append(
            {
                "x_cat": x_cat,
                "w_in": Win16,
                "w_hh": np.ascontiguousarray(whh_c),
                "w_out": wout_c,
                "bias0": bias0,
                "bias": np.ascontiguousarray(b16[1:4, cs:ce].reshape(1, 3 * SH)),
            }
        )
    return in_maps


def kernel(**inputs):
    global LAST_RESULTS
    import os

    from concourse import bass_utils

    if "nc" not in _CACHE:
        _CACHE["nc"] = _build_nc()
    nc = _CACHE["nc"]

    in_maps = _prep_inputs(**inputs)
    trace = bool(int(os.environ.get("BASS_TRACE_KERNEL", "0")))
    res = bass_utils.run_bass_kernel_spmd(
        nc, in_maps, core_ids=list(range(NCORES)), trace=trace
    )
    LAST_RESULTS = res
    partials = np.stack([r["out_partial"][0] for r in res.results])  # [8, 8]
    return partials.sum(axis=0).astype(np.float32)



# revision 7
# speedup vs baseline: 1.3100x; 1.3100x over previous
"""Trainium2 Bass kernel for a 4-layer dense MLP (H=8192), batch=1.

Tensor-parallel over 8 NeuronCores. Structure (per core):

  - Layer 1 (10x8192, replicated) computes straight into the [128, 64]
    chunked activation layout: 64 matmuls with [11, 128] stationary
    slices of an augmented (W_in | bias0) matrix and moving x_aug
    ([x; s; 1]), 8 columns per PSUM tile, sigmoid straight to fp8.
    No DRAM bounce; layer 2 starts on the first chunk immediately.

  - Hidden layers 2-4 are column-sharded (core c owns 1024 columns),
    each computed as two 512-column halves A/B so the AllGather of half
    A overlaps half B's compute, and the next layer's contraction is
    ordered gathered-A-first (host-side weight-row permutation).

  - Weight precision per layer (the problem is HBM-bound; fp8 halves
    the 48 MiB/core fp16 stream):
      L2: fp8 e4m3 (x 2^13) with DoubleRow perf mode (activations a1
          quantized to e4m3; 2x PE moving-ingestion rate).
      L3, L4: fp8 e3m4 (x 2^9, one extra mantissa bit) with fp16
          activations. Descale is folded into the sigmoid's scale arg.
    Measured end-to-end error vs the f32 reference: ~2e-3 (max-abs /
    max-abs-ref), PE time ~68us, DMA ~24.4 MiB -> ~74us.

  - Output layer (8192x8) is row-sharded: each core emits a partial
    [8]; the host sums. A dummy AllGather at kernel start absorbs the
    one-time ncfw rendezvous barrier.

Weights stream as 1 MiB DMAs (8 KiB per partition line) into
[128, 16, 512] SBUF tiles, 16 in flight.
"""

import numpy as np

H = 8192
D = 10  # input layer size (4 + 6)
DA = D + 1  # augmented with the bias row
OUT = 8
NCORES = 8
SH = H // NCORES  # 1024 columns per core
HF = 512  # half-width
KC = 64  # contraction chunks of 128 rows per half
GC = 16  # chunks per DMA group (1 MiB per DMA)
G = KC // GC  # 4 groups per half
WBUFS = 16  # in-flight weight DMA buffers (16 MiB SBUF)
S_L2 = float(2**13)  # e4m3 weight scale (|W|max*2^13 ~ 157 < 240)
S_L34 = float(2**9)  # e3m4 weight scale (|W|max*2^9 ~ 9.8 < 15.5)

LAST_RESULTS = None
_CACHE = {}


def _perm_ab():
    """Layer-3/4 input layout: chunks 0-31 hold the gathered A-halves
    (columns [0,512) of every rank), chunks 32-63 the B-halves.
    a_sb[p, k] = half_flat[p*32 + k%32] with half = k//32, and
    half_flat[i] = a_full[(i//512)*1024 + 512*half + i%512]."""
    k = np.arange(KC)[:, None]
    p = np.arange(128)[None, :]
    half = k // 32
    i = p * 32 + (k % 32)
    return (i // 512) * 1024 + 512 * half + (i % 512)


def _build_nc():
    import concourse.bacc as bacc
    import concourse.mybir as mybir
    import concourse.tile as tile

    f16 = mybir.dt.float16
    f32 = mybir.dt.float32
    f8e4 = mybir.dt.float8e4
    f8e3 = mybir.dt.float8e3
    SIG = mybir.ActivationFunctionType.Sigmoid
    DR = mybir.MatmulPerfMode.DoubleRow
    RG = [list(range(NCORES))]

    nc = bacc.Bacc(
        "TRN2", target_bir_lowering=False, debug=False, num_devices=NCORES
    )

    x_d = nc.dram_tensor("x_aug", [DA, 1], f16, kind="ExternalInput")
    win_d = nc.dram_tensor("w_in", [DA, H], f16, kind="ExternalInput")
    w2_d = nc.dram_tensor("w_l2", [2, G, 128, GC, HF], f8e4, kind="ExternalInput")
    w3_d = nc.dram_tensor("w_l3", [2, G, 128, GC, HF], f8e3, kind="ExternalInput")
    w4_d = nc.dram_tensor("w_l4", [2, G, 128, GC, HF], f8e3, kind="ExternalInput")
    wout_d = nc.dram_tensor("w_out", [128, 8 * OUT], f16, kind="ExternalInput")
    bias_d = nc.dram_tensor("bias", [1, 3 * SH], f16, kind="ExternalInput")
    out_d = nc.dram_tensor("out_partial", [1, OUT], f32, kind="ExternalOutput")

    with tile.TileContext(nc) as tc:
        with (
            tc.tile_pool(name="const", bufs=1) as cp,
            tc.tile_pool(name="wpool", bufs=WBUFS) as wp,
            tc.tile_pool(name="apool", bufs=2) as ap,
            tc.tile_pool(name="pspool", bufs=2, space="PSUM") as pp,
            tc.tile_pool(name="dpool", bufs=2, space="DRAM") as dp,
        ):
            one_sb = cp.tile([1, 1], f16)
            nc.gpsimd.memset(one_sb[:], 1.0)

            # Dummy collective: absorbs the one-time ncfw rendezvous
            # barrier concurrently with layer-1 compute + weight prefetch.
            warm_sb = cp.tile([1, 16], f16)
            nc.gpsimd.memset(warm_sb[:], 0.0)
            warm_in = dp.tile([1, 16], f16, tag="warmin")
            warm_out = dp.tile([8, 16], f16, tag="warmout")
            nc.gpsimd.dma_start(warm_in[:], warm_sb[:])
            nc.gpsimd.collective_compute(
                "AllGather",
                mybir.AluOpType.bypass,
                replica_groups=RG,
                ins=[warm_in.opt()],
                outs=[warm_out.opt()],
            )

            x_sb = cp.tile([DA, 1], f16)
            nc.scalar.dma_start(x_sb[:], x_d[:])
            win_sb = cp.tile([DA, H], f16)
            nc.scalar.dma_start(win_sb[:], win_d[:])
            bias_sb = cp.tile([1, 3 * SH], f16)
            nc.scalar.dma_start(bias_sb[:], bias_d[:])
            wout_sb = cp.tile([128, 8 * OUT], f16)
            nc.scalar.dma_start(wout_sb[:], wout_d[:])

            # ---- Layer 1, replicated: straight into the [128, 2, 32]
            # chunk-major layout (a8_sb[p, i, c] = a1[(i*32+c)*128 + p]).
            # The (2, 32) split gives DoubleRow lhsT pairs a 32 B pair
            # stride (ISA wants even + 16B-aligned), pairing chunk c
            # with chunk 32+c; the L2 weight row-perm matches. ----
            a8_sb = ap.tile([128, 2, KC // 2], f8e4, tag="a8")
            for j8 in range(8):
                hi, c0 = j8 // 4, (8 * j8) % 32
                ps1 = pp.tile([128, 1, 8], f32, tag="psL1")
                for jj in range(8):
                    j = 8 * j8 + jj
                    nc.tensor.matmul(
                        ps1[:, 0:1, jj : jj + 1],
                        win_sb[:, 128 * j : 128 * j + 128],
                        x_sb[:],
                        start=True,
                        stop=True,
                    )
                nc.scalar.activation(
                    a8_sb[:, hi : hi + 1, c0 : c0 + 8], ps1[:], SIG
                )

            # ---- Hidden layers 2-4: two 512-col halves, AG-A at mid-layer ----
            layers = [
                (w2_d, DR, 1.0 / S_L2),
                (w3_d, None, 1.0 / S_L34),
                (w4_d, None, 1.0 / S_L34),
            ]
            a_sb = a8_sb
            act_half = [None, None]
            for li, (w_d, pm, descale) in enumerate(layers):
                a_next = (
                    ap.tile([128, KC], f16, tag=f"a_l{li + 3}", name="a_next")
                    if li < 2
                    else None
                )
                for hf in range(2):
                    ps = pp.tile([1, HF], f32, tag=f"ps{hf}")
                    for g in range(G):
                        wt = wp.tile(
                            [128, GC, HF], f8e4 if pm is DR else f8e3, tag="w"
                        )
                        nc.sync.dma_start(wt[:], w_d[hf, g])
                        if pm is DR:
                            for c in range(GC // 2):
                                k = g * GC + 2 * c
                                cg = k // 2  # pair index
                                nc.tensor.matmul(
                                    ps[:],
                                    a_sb[:, :, cg : cg + 1],
                                    wt[:, 2 * c : 2 * c + 2, :],
                                    start=(k == 0),
                                    stop=False,
                                    perf_mode=DR,
                                )
                        else:
                            for c in range(GC):
                                k = g * GC + c
                                nc.tensor.matmul(
                                    ps[:],
                                    a_sb[:, k : k + 1],
                                    wt[:, c : c + 1, :],
                                    start=(k == 0),
                                    stop=False,
                                )
                    nc.tensor.matmul(
                        ps[:],
                        one_sb[:],
                        bias_sb[:, li * SH + hf * HF : li * SH + hf * HF + HF],
                        start=False,
                        stop=True,
                    )
                    act_h = ap.tile([1, HF], f16, tag=f"act{hf}")
                    nc.scalar.activation(act_h[:], ps[:], SIG, scale=descale)
                    act_half[hf] = act_h
                    if li < 2:
                        cc_in = dp.tile([1, HF], f16, tag=f"ccin{hf}")
                        cc_out = dp.tile([128, 32], f16, tag=f"ccout{hf}")
                        nc.gpsimd.dma_start(cc_in[:], act_h[:])
                        nc.gpsimd.collective_compute(
                            "AllGather",
                            mybir.AluOpType.bypass,
                            replica_groups=RG,
                            ins=[cc_in.opt()],
                            outs=[cc_out.opt()],
                        )
                        nc.scalar.dma_start(
                            a_next[:, 32 * hf : 32 * hf + 32], cc_out[:]
                        )
                if li < 2:
                    a_sb = a_next

            # ---- Output layer: row-sharded, partial [8] per core ----
            sc = dp.tile([1, SH], f16, tag="sc")
            nc.scalar.dma_start(sc[:, 0:HF], act_half[0][:])
            nc.scalar.dma_start(sc[:, HF:SH], act_half[1][:])
            a2_sb = ap.tile([128, 8], f16, tag="a2")
            nc.scalar.dma_start(
                a2_sb[:], sc.rearrange("one (p k) -> (one p) k", p=128)
            )
            pso = pp.tile([1, OUT], f32, tag="psO", bufs=1)
            for k in range(8):
                nc.tensor.matmul(
                    pso[:],
                    a2_sb[:, k : k + 1],
                    wout_sb[:, k * OUT : (k + 1) * OUT],
                    start=(k == 0),
                    stop=(k == 7),
                )
            res_sb = ap.tile([1, OUT], f32, tag="res")
            nc.vector.tensor_copy(res_sb[:], pso[:])
            nc.scalar.dma_start(out_d[:], res_sb[:])

    nc.compile()
    return nc


def _pack_layer(wcol_q, perm):
    """[8192, 1024] quantized core shard -> [2 halves, G, 128, GC, HF],
    rows permuted so chunk k, partition p holds row perm[k, p]."""
    wperm = wcol_q[perm]  # [KC, 128, 1024]
    grp = wperm.reshape(G, GC, 128, 2 * HF).transpose(0, 2, 1, 3)  # [G,128,GC,1024]
    return np.stack([grp[..., :HF], grp[..., HF:]])  # [2, G, 128, GC, HF]


def _prep_inputs(x, s, W_in, W_hh, W_out, b):
    """Shard + quantize + lay out the inputs for each of the 8 cores."""
    import ml_dtypes

    f16 = np.float16
    e4 = ml_dtypes.float8_e4m3
    e3 = ml_dtypes.float8_e3m4

    x_aug = np.concatenate(
        [np.asarray(x), np.asarray(s), np.ones(1, np.float32)]
    ).astype(f16)
    x_aug = np.ascontiguousarray(x_aug.reshape(DA, 1))
    b32 = np.asarray(b, np.float32)  # [5, 8192] (b[4] unused)
    win_aug = np.ascontiguousarray(
        np.concatenate([np.asarray(W_in), b32[0:1]], axis=0).astype(f16)
    )  # [11, 8192]
    Whh = np.asarray(W_hh, np.float32)  # [3, 8192, 8192]
    Wout16 = np.asarray(W_out).astype(f16)  # [8192, 8]

    # chunk k, partition p -> row. L2 (DoubleRow): weight block b pairs
    # with a8_sb column q = (b%2)*32 + b//2 holding rows q*128 + p.
    # L3/4: AG output layout.
    k = np.arange(KC)[:, None]
    p = np.arange(128)[None, :]
    perm_l2 = ((k % 2) * 32 + k // 2) * 128 + p
    perm_ab = _perm_ab()

    # host-scaled biases (zeros in this problem, but kept faithful)
    bias_rows = np.concatenate(
        [b32[1] * S_L2, b32[2] * S_L34, b32[3] * S_L34]
    ).astype(f16)  # [3*8192]

    in_maps = []
    for c in range(NCORES):
        cs, ce = c * SH, (c + 1) * SH
        w2 = _pack_layer(
            (Whh[0][:, cs:ce] * S_L2).astype(e4), perm_l2
        )
        w3 = _pack_layer(
            (Whh[1][:, cs:ce] * S_L34).astype(e3), perm_ab
        )
        w4 = _pack_layer(
            (Whh[2][:, cs:ce] * S_L34).astype(e3), perm_ab
        )
        bias_c = np.concatenate(
            [bias_rows[li * H + cs : li * H + ce] for li in range(3)]
        ).reshape(1, 3 * SH)
        in_maps.append(
            {
                "x_aug": x_aug,
                "w_in": win_aug,
                "w_l2": np.ascontiguousarray(w2),
                "w_l3": np.ascontiguousarray(w3),
                "w_l4": np.ascontiguousarray(w4),
                "w_out": np.ascontiguousarray(Wout16[cs:ce].reshape(128, 8 * OUT)),
                "bias": np.ascontiguousarray(bias_c),
            }
        )
    return in_maps


def kernel(**inputs):
    global LAST_RESULTS
    import os

    from concourse import bass_utils

    if "nc" not in _CACHE:
        _CACHE["nc"] = _build_nc()
    nc = _CACHE["nc"]

    in_maps = _prep_inputs(**inputs)
    trace = bool(int(os.environ.get("BASS_TRACE_KERNEL", "0")))
    res = bass_utils.run_bass_kernel_spmd(
        nc, in_maps, core_ids=list(range(NCORES)), trace=trace
    )
    LAST_RESULTS = res
    partials = np.stack([r["out_partial"][0] for r in res.results])  # [8, 8]
    return partials.sum(axis=0).astype(np.float32)


# revision 10
# speedup vs baseline: 1.3174x; 1.0056x over previous
"""Trainium2 Bass kernel for a 4-layer dense MLP (H=8192), batch=1.

Tensor-parallel over 8 NeuronCores. Structure (per core):

  - Layer 1 (10x8192, replicated) computes straight into the [128, 2, 32]
    chunked activation layout: 64 matmuls with [11, 128] stationary
    slices of an augmented (W_in | bias0) matrix and moving x_aug
    ([x; s; 1]), sigmoid straight to fp8. No DRAM bounce.

  - Hidden layers 2-4 are column-sharded (core c owns 1024 columns),
    each computed as two 512-col output halves A/B so the AllGather of
    half A overlaps half B's compute; the next layer's contraction is
    ordered gathered-A-first (host-side weight-row permutations).

  - AllGather outputs ([8 ranks x 512] f16 in DRAM) are unpacked with a
    contiguous [8, 512] SBUF load + 4 PE-transpose ops (identity rhs)
    instead of a 128-line scatter DMA — the scatter cost ~10us per
    boundary on the critical path.

  - Weight precision (the problem is HBM-bound; fp8 halves the 48
    MiB/core fp16 stream): L2, L3 fp8 e4m3 (x 2^13) with DoubleRow perf
    mode (2x PE ingestion; their input activations quantize to e4m3);
    L4 fp8 e3m4 (x 2^9, extra mantissa bit) with fp16 activations.
    Descales fold into the sigmoid's scale argument. Host-sim error vs
    the f32 reference: ~4.7e-3 (max-abs / max-abs-ref).

  - Output layer (8192x8) row-sharded: activations transposed into
    [128, 8] via PE transposes (no DRAM bounce), partial [8] per core,
    host sums. A dummy AllGather (shaped like the real ones so the mesh
    algo setup is warmed too) absorbs the one-time ncfw rendezvous.

Weights stream as 1 MiB DMAs (8 KiB per partition line) into
[128, 16, 512] SBUF tiles, 16 in flight: L3+L4 fully buffered during
the collective phase, so the post-barrier chain is pure PE + AG.
"""

import numpy as np

H = 8192
D = 10  # input layer size (4 + 6)
DA = D + 1  # augmented with the bias row
OUT = 8
NCORES = 8
SH = H // NCORES  # 1024 columns per core
HF = 512  # half-width
KC = 64  # contraction chunks of 128 rows
GC = 16  # chunks per DMA group (1 MiB per DMA)
G = KC // GC  # 4 groups per output half
WBUFS = 16  # in-flight weight DMA buffers (16 MiB SBUF)
S_DR = float(2**13)  # e4m3 weight scale (|W|max*2^13 ~ 157 < 240)
S_E3 = float(2**9)  # e3m4 weight scale (|W|max*2^9 ~ 9.8 < 15.5)

LAST_RESULTS = None
_CACHE = {}


def _build_nc():
    import concourse.bacc as bacc
    import concourse.mybir as mybir
    import concourse.tile as tile

    f16 = mybir.dt.float16
    f32 = mybir.dt.float32
    f8e4 = mybir.dt.float8e4
    f8e3 = mybir.dt.float8e3
    SIG = mybir.ActivationFunctionType.Sigmoid
    DR = mybir.MatmulPerfMode.DoubleRow
    RG = [list(range(NCORES))]

    nc = bacc.Bacc(
        "TRN2", target_bir_lowering=False, debug=False, num_devices=NCORES
    )

    x_d = nc.dram_tensor("x_aug", [DA, 1], f16, kind="ExternalInput")
    win_d = nc.dram_tensor("w_in", [DA, H], f16, kind="ExternalInput")
    w2_d = nc.dram_tensor("w_l2", [2, G, 128, GC, HF], f8e4, kind="ExternalInput")
    w3_d = nc.dram_tensor("w_l3", [2, G, 128, GC, HF], f8e4, kind="ExternalInput")
    w4_d = nc.dram_tensor("w_l4", [2, G, 128, GC, HF], f8e3, kind="ExternalInput")
    wout_d = nc.dram_tensor("w_out", [128, 8 * OUT], f16, kind="ExternalInput")
    bias_d = nc.dram_tensor("bias", [1, 3 * SH], f16, kind="ExternalInput")
    id_d = nc.dram_tensor("ident", [8, 8], f16, kind="ExternalInput")
    out_d = nc.dram_tensor("out_partial", [1, OUT], f32, kind="ExternalOutput")

    with tile.TileContext(nc) as tc:
        with (
            tc.tile_pool(name="const", bufs=1) as cp,
            tc.tile_pool(name="wpool", bufs=WBUFS) as wp,
            tc.tile_pool(name="apool", bufs=2) as ap,
            tc.tile_pool(name="pspool", bufs=2, space="PSUM") as pp,
            tc.tile_pool(name="dpool", bufs=2, space="DRAM") as dp,
        ):
            one_sb = cp.tile([1, 1], f16)
            nc.gpsimd.memset(one_sb[:], 1.0)

            # Dummy collective, same shape as the real ones: absorbs the
            # one-time ncfw rendezvous + mesh algo setup concurrently
            # with layer-1 compute + weight prefetch.
            warm_sb = cp.tile([1, HF], f16)
            nc.gpsimd.memset(warm_sb[:], 0.0)
            warm_in = dp.tile([1, HF], f16, tag="warmin")
            warm_out = dp.tile([8, HF], f16, tag="warmout")
            nc.gpsimd.dma_start(warm_in[:], warm_sb[:])
            nc.gpsimd.collective_compute(
                "AllGather",
                mybir.AluOpType.bypass,
                replica_groups=RG,
                ins=[warm_in.opt()],
                outs=[warm_out.opt()],
            )

            x_sb = cp.tile([DA, 1], f16)
            nc.scalar.dma_start(x_sb[:], x_d[:])
            win_sb = cp.tile([DA, H], f16)
            nc.scalar.dma_start(win_sb[:], win_d[:])
            bias_sb = cp.tile([1, 3 * SH], f16)
            nc.scalar.dma_start(bias_sb[:], bias_d[:])
            wout_sb = cp.tile([128, 8 * OUT], f16)
            nc.scalar.dma_start(wout_sb[:], wout_d[:])
            ident_sb = cp.tile([8, 8], f16)
            nc.scalar.dma_start(ident_sb[:], id_d[:])

            # ---- Layer 1, replicated: straight into the [128, 2, 32]
            # layout (a8_sb[p, i, c] = a1[(i*32+c)*128 + p]); the (2, 32)
            # split gives DoubleRow lhsT pairs a 32 B pair stride. ----
            a8_sb = ap.tile([128, 2, KC // 2], f8e4, tag="a8")
            for j8 in range(8):
                hi, c0 = j8 // 4, (8 * j8) % 32
                ps1 = pp.tile([128, 1, 8], f32, tag="psL1", bufs=1)
                for jj in range(8):
                    j = 8 * j8 + jj
                    nc.tensor.matmul(
                        ps1[:, 0:1, jj : jj + 1],
                        win_sb[:, 128 * j : 128 * j + 128],
                        x_sb[:],
                        start=True,
                        stop=True,
                    )
                nc.scalar.activation(
                    a8_sb[:, hi : hi + 1, c0 : c0 + 8], ps1[:], SIG
                )

            # ---- helpers ----
            def emit_gather(act_h, hf):
                """AllGather one 512-col output half; returns cc_out."""
                cc_in = dp.tile([1, HF], f16, tag=f"ccin{hf}")
                cc_out = dp.tile([8, HF], f16, tag=f"ccout{hf}")
                nc.gpsimd.dma_start(cc_in[:], act_h[:])
                nc.gpsimd.collective_compute(
                    "AllGather",
                    mybir.AluOpType.bypass,
                    replica_groups=RG,
                    ins=[cc_in.opt()],
                    outs=[cc_out.opt()],
                )
                return cc_out

            def emit_unpack(cc_out, a_dst, hf_in, dr):
                """[8, 512] gathered half -> 4 PE transposes -> a_dst.
                Column q=8j+r of the transpose holds rank r's cols
                [128j, 128j+128) of this half."""
                g8 = ap.tile([8, HF], f16, tag="g8", name="g8")
                nc.scalar.dma_start(g8[:], cc_out[:])
                if dr:
                    psT = pp.tile([128, 2, 16], f16, tag="psT3", bufs=1)
                    for j in range(4):
                        nc.tensor.matmul(
                            psT[:, j // 2 : j // 2 + 1, 8 * (j % 2) : 8 * (j % 2) + 8],
                            g8[:, 128 * j : 128 * j + 128],
                            ident_sb[:],
                            is_transpose=True,
                            start=True,
                            stop=True,
                        )
                    for i in range(2):
                        nc.vector.tensor_copy(
                            a_dst[:, i : i + 1, 16 * hf_in : 16 * hf_in + 16],
                            psT[:, i : i + 1, :],
                        )
                else:
                    psT = pp.tile([128, 32], f16, tag="psT4", bufs=1)
                    for j in range(4):
                        nc.tensor.matmul(
                            psT[:, 8 * j : 8 * j + 8],
                            g8[:, 128 * j : 128 * j + 128],
                            ident_sb[:],
                            is_transpose=True,
                            start=True,
                            stop=True,
                        )
                    nc.vector.tensor_copy(
                        a_dst[:, 32 * hf_in : 32 * hf_in + 32], psT[:]
                    )

            def emit_hidden(w_d, pm, descale, a_in, bias_off, inject_b=None):
                """One hidden layer: 2 output halves x 4 weight groups.
                inject_b() is called before group 2 of half 0 — the spot
                where the previous boundary's B-half unpack goes (its
                AG has landed by then; groups 0-1 touch only A data)."""
                outs = []
                for hf in range(2):
                    ps = pp.tile([1, HF], f32, tag="psH", bufs=2, name="ps")
                    for g in range(G):
                        if inject_b is not None and hf == 0 and g == 2:
                            inject_b()
                        wt = wp.tile(
                            [128, GC, HF],
                            f8e4 if pm is DR else f8e3,
                            tag="w",
                            name="wt",
                        )
                        nc.sync.dma_start(wt[:], w_d[hf, g])
                        if pm is DR:
                            for c in range(GC // 2):
                                k = g * GC + 2 * c
                                nc.tensor.matmul(
                                    ps[:],
                                    a_in[:, :, k // 2 : k // 2 + 1],
                                    wt[:, 2 * c : 2 * c + 2, :],
                                    start=(k == 0),
                                    stop=False,
                                    perf_mode=DR,
                                )
                        else:
                            for c in range(GC):
                                k = g * GC + c
                                nc.tensor.matmul(
                                    ps[:],
                                    a_in[:, k : k + 1],
                                    wt[:, c : c + 1, :],
                                    start=(k == 0),
                                    stop=False,
                                )
                    nc.tensor.matmul(
                        ps[:],
                        one_sb[:],
                        bias_sb[:, bias_off + hf * HF : bias_off + hf * HF + HF],
                        start=False,
                        stop=True,
                    )
                    act_h = ap.tile([1, HF], f16, tag=f"act{hf}", name="act_h")
                    nc.scalar.activation(act_h[:], ps[:], SIG, scale=descale)
                    outs.append(act_h)
                return outs

            # ---- Layer 2 (DoubleRow e4m3) ----
            act = emit_hidden(w2_d, DR, 1.0 / S_DR, a8_sb, 0)
            cc_a = emit_gather(act[0], 0)
            cc_b = emit_gather(act[1], 1)

            # ---- Layer 3 (DoubleRow e4m3): input a3 [128, 2, 32] fp8 ----
            a3_sb = ap.tile([128, 2, KC // 2], f8e4, tag="a3")
            emit_unpack(cc_a, a3_sb, 0, dr=True)
            ccb = cc_b

            def inject3(cc=ccb):
                emit_unpack(cc, a3_sb, 1, dr=True)

            act = emit_hidden(w3_d, DR, 1.0 / S_DR, a3_sb, SH, inject_b=inject3)
            cc_a = emit_gather(act[0], 0)
            cc_b = emit_gather(act[1], 1)

            # ---- Layer 4 (plain e3m4): input a4 [128, 64] f16 ----
            a4_sb = ap.tile([128, KC], f16, tag="a4")
            emit_unpack(cc_a, a4_sb, 0, dr=False)
            ccb2 = cc_b

            def inject4(cc=ccb2):
                emit_unpack(cc, a4_sb, 1, dr=False)

            act = emit_hidden(w4_d, None, 1.0 / S_E3, a4_sb, 2 * SH, inject_b=inject4)

            # ---- Output layer: transpose acts to [128, 8], row-sharded
            # partial [8] per core (a2_sb[p, t] = act_local[128t + p]) ----
            psOT = pp.tile([128, 8, 2], f16, tag="psOT", bufs=1)
            for t in range(8):
                hf, off = t // 4, 128 * (t % 4)
                nc.tensor.matmul(
                    psOT[:, t : t + 1, 0:1],
                    act[hf][:, off : off + 128],
                    one_sb[:],
                    is_transpose=True,
                    start=True,
                    stop=True,
                )
            a2_sb = ap.tile([128, 8, 2], f16, tag="a2")
            nc.vector.tensor_copy(a2_sb[:], psOT[:])
            pso = pp.tile([1, OUT], f32, tag="psO", bufs=1)
            for t in range(8):
                nc.tensor.matmul(
                    pso[:],
                    a2_sb[:, t : t + 1, 0:1],
                    wout_sb[:, t * OUT : (t + 1) * OUT],
                    start=(t == 0),
                    stop=(t == 7),
                )
            res_sb = ap.tile([1, OUT], f32, tag="res")
            nc.vector.tensor_copy(res_sb[:], pso[:])
            nc.scalar.dma_start(out_d[:], res_sb[:])

    nc.compile()
    return nc


def _pack_layer(wcol_q, perm):
    """[8192, 1024] quantized core shard -> [2 halves, G, 128, GC, HF],
    rows permuted so weight block b, partition p holds row perm[b, p]."""
    wperm = wcol_q[perm]  # [KC, 128, 1024]
    grp = wperm.reshape(G, GC, 128, 2 * HF).transpose(0, 2, 1, 3)  # [G,128,GC,1024]
    return np.stack([grp[..., :HF], grp[..., HF:]])  # [2, G, 128, GC, HF]


def _prep_inputs(x, s, W_in, W_hh, W_out, b):
    """Shard + quantize + lay out the inputs for each of the 8 cores."""
    import ml_dtypes

    f16 = np.float16
    e4 = ml_dtypes.float8_e4m3
    e3 = ml_dtypes.float8_e3m4

    x_aug = np.concatenate(
        [np.asarray(x), np.asarray(s), np.ones(1, np.float32)]
    ).astype(f16)
    x_aug = np.ascontiguousarray(x_aug.reshape(DA, 1))
    b32 = np.asarray(b, np.float32)  # [5, 8192] (b[4] unused)
    win_aug = np.ascontiguousarray(
        np.concatenate([np.asarray(W_in), b32[0:1]], axis=0).astype(f16)
    )  # [11, 8192]
    Whh = np.asarray(W_hh, np.float32)  # [3, 8192, 8192]
    Wout32 = np.asarray(W_out, np.float32)  # [8192, 8]

    # weight block b (or chunk k), partition p -> global activation row.
    bb = np.arange(KC)[:, None]
    p = np.arange(128)[None, :]
    # L2 (DR): block b pairs with a8 col q=(b%2)*32+b//2 = rows q*128+p.
    perm_l2 = (((bb % 2) * 32 + bb // 2) * 128) + p
    # L3 (DR): pair c=b//2 (c<16: A half), i=b%2; in-half chunk
    # q=16i+(c%16); row = (q%8)*1024 + (c//16)*512 + (q//8)*128 + p.
    c_, i_ = bb // 2, bb % 2
    q_ = 16 * i_ + (c_ % 16)
    perm_l3 = (q_ % 8) * 1024 + (c_ // 16) * 512 + (q_ // 8) * 128 + p
    # L4 (plain): chunk k (k<32: A half), q=k%32;
    # row = (q%8)*1024 + (k//32)*512 + (q//8)*128 + p.
    q4 = bb % 32
    perm_l4 = (q4 % 8) * 1024 + (bb // 32) * 512 + (q4 // 8) * 128 + p

    bias_rows = np.concatenate(
        [b32[1] * S_DR, b32[2] * S_DR, b32[3] * S_E3]
    ).astype(f16)  # [3*8192], host-scaled (zeros in this problem)

    ident = np.eye(8, dtype=f16)

    in_maps = []
    for c in range(NCORES):
        cs, ce = c * SH, (c + 1) * SH
        w2 = _pack_layer((Whh[0][:, cs:ce] * S_DR).astype(e4), perm_l2)
        w3 = _pack_layer((Whh[1][:, cs:ce] * S_DR).astype(e4), perm_l3)
        w4 = _pack_layer((Whh[2][:, cs:ce] * S_E3).astype(e3), perm_l4)
        bias_c = np.concatenate(
            [bias_rows[li * H + cs : li * H + ce] for li in range(3)]
        ).reshape(1, 3 * SH)
        # out layer: a2_sb[p, t] = act_local[128t + p]
        wout_c = np.ascontiguousarray(
            Wout32[cs:ce].reshape(8, 128, OUT).transpose(1, 0, 2)
            .reshape(128, 8 * OUT).astype(f16)
        )
        in_maps.append(
            {
                "x_aug": x_aug,
                "w_in": win_aug,
                "w_l2": np.ascontiguousarray(w2),
                "w_l3": np.ascontiguousarray(w3),
                "w_l4": np.ascontiguousarray(w4),
                "w_out": wout_c,
                "bias": np.ascontiguousarray(bias_c),
                "ident": ident,
            }
        )
    return in_maps


def kernel(**inputs):
    global LAST_RESULTS
    import os

    from concourse import bass_utils

    if "nc" not in _CACHE:
        _CACHE["nc"] = _build_nc()
    nc = _CACHE["nc"]

    in_maps = _prep_inputs(**inputs)
    trace = bool(int(os.environ.get("BASS_TRACE_KERNEL", "0")))
    res = bass_utils.run_bass_kernel_spmd(
        nc, in_maps, core_ids=list(range(NCORES)), trace=trace
    )
    LAST_RESULTS = res
    partials = np.stack([r["out_partial"][0] for r in res.results])  # [8, 8]
    return partials.sum(axis=0).astype(np.float32)
